# revision 3
# baseline (speedup 1.0000x reference)
"""Grouped GEMM (MoE routing) Trainium2 kernel.

Expert-parallel across 8 NeuronCores: core c owns experts [8c, 8c+8). Since
tokens are laid out contiguously by expert, each core's tokens are a
contiguous row-slice of the input, computed host-side from tokens_per_expert.

Per expert: out[CAP,DOUT] = x[CAP,DIN] @ w[DIN,DOUT], CAP=256-padded so one
SPMD program serves all cores. PE matmuls run in float32r (full-rate fp32,
~1.5e-4 rel err). Weights stream through SBUF in [128, DOUT] K-slabs; each
slab feeds 8 accumulating PSUM tiles (2 token-tiles x 4 DOUT-chunks).
"""
import numpy as np

import concourse.bass as bass
import concourse.mybir as mybir
import concourse.tile as tile
from concourse import bacc
from concourse.bass_utils import run_bass_kernel_spmd

G, T, DIN, DOUT, CAP = 64, 8192, 2560, 1664, 256
NCORES = 8
EPC = G // NCORES   # experts per core
KC = DIN // 128     # 20 contraction chunks
NT = 4              # DOUT chunks
NW = DOUT // NT     # 416 (<=512 fp32 PSUM bank, >=256 for full-rate f32r)
MT = CAP // 128     # 2 token tiles per expert

_cache = {}


def _build():
    nc = bacc.Bacc(trn_type="TRN2", debug=False)
    f32r = mybir.dt.float32r
    xt = nc.dram_tensor("xt", [EPC, DIN, CAP], f32r, kind="ExternalInput").ap()
    w = nc.dram_tensor("w", [EPC, DIN, DOUT], f32r, kind="ExternalInput").ap()
    out = nc.dram_tensor(
        "out", [EPC, CAP, DOUT], mybir.dt.float32, kind="ExternalOutput"
    ).ap()
    with tile.TileContext(nc) as tc:
        with (
            tc.tile_pool(name="xtp", bufs=2) as xt_pool,
            tc.tile_pool(name="wp", bufs=4) as w_pool,
            tc.tile_pool(name="op", bufs=4) as o_pool,
            tc.tile_pool(name="ps", bufs=1, space="PSUM") as ps_pool,
        ):
            for e in range(EPC):
                xt_sb = xt_pool.tile([128, KC * CAP], f32r, tag="xt")
                nc.sync.dma_start(
                    xt_sb[:].rearrange("p (c t) -> p c t", c=KC),
                    xt[e].rearrange("(c p) t -> p c t", p=128),
                )
                psums = {}
                for m in range(MT):
                    for n in range(NT):
                        psums[m, n] = ps_pool.tile(
                            [128, NW], mybir.dt.float32, tag=f"ps{m}{n}", name=f"psum_{m}_{n}"
                        )
                for k in range(KC):
                    w_sb = w_pool.tile([128, DOUT], f32r, tag="w")
                    nc.sync.dma_start(w_sb[:], w[e, k * 128:(k + 1) * 128, :])
                    for m in range(MT):
                        for n in range(NT):
                            nc.tensor.matmul(
                                psums[m, n][:],
                                xt_sb[:, k * CAP + m * 128: k * CAP + (m + 1) * 128],
                                w_sb[:, n * NW:(n + 1) * NW],
                                start=(k == 0),
                                stop=(k == KC - 1),
                            )
                for m in range(MT):
                    o_sb = o_pool.tile([128, DOUT], mybir.dt.float32, tag="o")
                    for n in range(NT):
                        nc.vector.tensor_copy(
                            o_sb[:, n * NW:(n + 1) * NW], psums[m, n][:]
                        )
                    nc.scalar.dma_start(out[e, m * 128:(m + 1) * 128, :], o_sb[:])
    nc.compile()
    return nc


def _run(inputs, trace=False):
    x = np.asarray(inputs["input"], dtype=np.float32)
    w = np.ascontiguousarray(np.asarray(inputs["weight"], dtype=np.float32))
    counts = np.asarray(inputs["tokens_per_expert"], dtype=np.int64)
    starts = np.concatenate([[0], np.cumsum(counts)[:-1]])

    xt_all = np.zeros((G, DIN, CAP), dtype=np.float32)
    for g in range(G):
        c = int(counts[g])
        if c:
            xt_all[g, :, :c] = x[starts[g]:starts[g] + c].T

    if "nc" not in _cache:
        _cache["nc"] = _build()
    nc = _cache["nc"]

    in_maps = [
        {"xt": xt_all[c * EPC:(c + 1) * EPC], "w": w[c * EPC:(c + 1) * EPC]}
        for c in range(NCORES)
    ]
    kw = {"trace_cores": list(range(NCORES))} if trace else {}
    res = run_bass_kernel_spmd(nc, in_maps, core_ids=list(range(NCORES)), trace=trace, **kw)

    out = np.empty((T, DOUT), dtype=np.float32)
    for g in range(G):
        c = int(counts[g])
        if c:
            out[starts[g]:starts[g] + c] = res.results[g // EPC]["out"][g % EPC, :c]
    return out, res


def kernel(**inputs) -> np.ndarray:
    return _run(inputs)[0]


# revision 4
# speedup vs baseline: 1.0965x; 1.0965x over previous
"""Grouped GEMM (MoE routing) Trainium2 kernel.

Expert-parallel across 8 NeuronCores with size-sorted slot assignment:
experts are sorted by token count and slot s on every core holds the
experts of size-rank [8s, 8s+8), so one SPMD program with per-slot
capacities cap_s = roundup32(max count in rank group) serves all cores
with ~40% less padding than a fixed CAP=256.

Per slot: out[cap_s, DOUT] = x[cap_s, DIN] @ w[DIN, DOUT] on the PE in
float32r (fast fp32 mode, ~1.5e-4 rel err), lhsT = host-transposed token
tiles, rhs = weight K-slabs [128, DOUT] streamed through SBUF,
accumulating over 20 K-chunks in PSUM ([128, 416] tiles, 4 DOUT chunks).
"""
import numpy as np

import concourse.bass as bass
import concourse.mybir as mybir
import concourse.tile as tile
from concourse import bacc
from concourse.bass_utils import run_bass_kernel_spmd

G, T, DIN, DOUT = 64, 8192, 2560, 1664
NCORES = 8
EPC = G // NCORES   # expert slots per core
KC = DIN // 128     # 20 contraction chunks
NT = 4              # DOUT chunks
NW = DOUT // NT     # 416 (<=512 fp32 PSUM bank, >=256 for full-rate f32r)

_cache = {}


def _build(caps):
    offs = np.concatenate([[0], np.cumsum(caps)]).astype(int)
    sumcap = int(offs[-1])
    nc = bacc.Bacc(trn_type="TRN2", debug=False)
    f32r = mybir.dt.float32r
    xt = nc.dram_tensor("xt", [DIN, sumcap], f32r, kind="ExternalInput").ap()
    w = nc.dram_tensor("w", [EPC, DIN, DOUT], f32r, kind="ExternalInput").ap()
    out = nc.dram_tensor(
        "out", [sumcap, DOUT], mybir.dt.float32, kind="ExternalOutput"
    ).ap()
    with tile.TileContext(nc) as tc:
        with (
            tc.tile_pool(name="xtp", bufs=2) as xt_pool,
            tc.tile_pool(name="wp", bufs=6) as w_pool,
            tc.tile_pool(name="op", bufs=4) as o_pool,
            tc.tile_pool(name="ps", bufs=1, space="PSUM") as ps_pool,
        ):
            for s in range(EPC):
                cap = int(caps[s])
                off = int(offs[s])
                mts = (cap + 127) // 128  # m-tiles in this slot
                xt_sb = xt_pool.tile([128, KC * cap], f32r, tag="xt", name=f"xt{s}")
                nc.sync.dma_start(
                    xt_sb[:].rearrange("p (c t) -> p c t", c=KC),
                    xt[:, off:off + cap].rearrange("(c p) t -> p c t", p=128),
                )
                psums = {}
                for m in range(mts):
                    for n in range(NT):
                        psums[m, n] = ps_pool.tile(
                            [128, NW], mybir.dt.float32, tag=f"ps{m}{n}",
                            name=f"psum_{s}_{m}_{n}",
                        )
                for k in range(KC):
                    w_sb = w_pool.tile([128, DOUT], f32r, tag="w", name=f"w{s}_{k}")
                    nc.sync.dma_start(w_sb[:], w[s, k * 128:(k + 1) * 128, :])
                    for m in range(mts):
                        msz = min(128, cap - m * 128)
                        for n in range(NT):
                            nc.tensor.matmul(
                                psums[m, n][:msz],
                                xt_sb[:, k * cap + m * 128: k * cap + m * 128 + msz],
                                w_sb[:, n * NW:(n + 1) * NW],
                                start=(k == 0),
                                stop=(k == KC - 1),
                            )
                for m in range(mts):
                    msz = min(128, cap - m * 128)
                    o_sb = o_pool.tile([128, DOUT], mybir.dt.float32, tag="o",
                                       name=f"o_{s}_{m}")
                    for n in range(NT):
                        nc.vector.tensor_copy(
                            o_sb[:msz, n * NW:(n + 1) * NW], psums[m, n][:msz]
                        )
                    nc.scalar.dma_start(
                        out[off + m * 128: off + m * 128 + msz, :], o_sb[:msz]
                    )
    nc.compile()
    return nc


def _run(inputs, trace=False):
    x = np.asarray(inputs["input"], dtype=np.float32)
    w = np.ascontiguousarray(np.asarray(inputs["weight"], dtype=np.float32))
    counts = np.asarray(inputs["tokens_per_expert"], dtype=np.int64)
    starts = np.concatenate([[0], np.cumsum(counts)[:-1]])

    order = np.argsort(-counts, kind="stable")  # experts by size rank
    # slot s, core c -> expert order[s*NCORES + c]; capacity = rank-group max
    caps = tuple(
        int(np.ceil(max(1, counts[order[s * NCORES:(s + 1) * NCORES]].max()) / 32) * 32)
        for s in range(EPC)
    )
    offs = np.concatenate([[0], np.cumsum(caps)]).astype(int)
    sumcap = int(offs[-1])

    if caps not in _cache:
        _cache[caps] = _build(caps)
    nc = _cache[caps]

    in_maps = []
    for c in range(NCORES):
        xt_pack = np.zeros((DIN, sumcap), dtype=np.float32)
        w_pack = np.empty((EPC, DIN, DOUT), dtype=np.float32)
        for s in range(EPC):
            g = int(order[s * NCORES + c])
            cnt = int(counts[g])
            if cnt:
                xt_pack[:, offs[s]:offs[s] + cnt] = x[starts[g]:starts[g] + cnt].T
            w_pack[s] = w[g]
        in_maps.append({"xt": xt_pack, "w": w_pack})

    kw = {"trace_cores": list(range(NCORES))} if trace else {}
    res = run_bass_kernel_spmd(nc, in_maps, core_ids=list(range(NCORES)),
                               trace=trace, **kw)

    out = np.empty((T, DOUT), dtype=np.float32)
    for c in range(NCORES):
        for s in range(EPC):
            g = int(order[s * NCORES + c])
            cnt = int(counts[g])
            if cnt:
                out[starts[g]:starts[g] + cnt] = \
                    res.results[c]["out"][offs[s]:offs[s] + cnt]
    return out, res


def kernel(**inputs) -> np.ndarray:
    return _run(inputs)[0]


# revision 5
# speedup vs baseline: 1.1357x; 1.0358x over previous
"""Grouped GEMM (MoE routing) Trainium2 kernel.

Expert-parallel across 8 NeuronCores with size-sorted slot assignment:
experts are sorted by token count and slot s on every core holds the
experts of size-rank [8s, 8s+8), so one SPMD program with per-slot
capacities cap_s = roundup32(max count in rank group) serves all cores
with ~40% less padding than a fixed CAP=256.

Per slot: out[cap_s, DOUT] = x[cap_s, DIN] @ w[DIN, DOUT] on the PE in
float32r (fast fp32 mode, ~1.5e-4 rel err), lhsT = host-transposed token
tiles, rhs = weight K-slabs [128, DOUT] streamed through SBUF,
accumulating over 20 K-chunks in PSUM ([128, 416] tiles, 4 DOUT chunks).
"""
import numpy as np

import concourse.bass as bass
import concourse.mybir as mybir
import concourse.tile as tile
from concourse import bacc
from concourse.bass_utils import run_bass_kernel_spmd

G, T, DIN, DOUT = 64, 8192, 2560, 1664
NCORES = 8
EPC = G // NCORES   # expert slots per core
KC = DIN // 128     # 20 contraction chunks
NT = 4              # DOUT chunks
NW = DOUT // NT     # 416 (<=512 fp32 PSUM bank, >=256 for full-rate f32r)

_cache = {}


def _build(caps):
    offs = np.concatenate([[0], np.cumsum(caps)]).astype(int)
    sumcap = int(offs[-1])
    nc = bacc.Bacc(trn_type="TRN2", debug=False)
    f32r = mybir.dt.float32r
    xt = nc.dram_tensor("xt", [DIN, sumcap], f32r, kind="ExternalInput").ap()
    w = nc.dram_tensor("w", [EPC, DIN, DOUT], f32r, kind="ExternalInput").ap()
    out = nc.dram_tensor(
        "out", [sumcap, DOUT], mybir.dt.float32, kind="ExternalOutput"
    ).ap()
    with tile.TileContext(nc) as tc:
        with (
            tc.tile_pool(name="xtp", bufs=2) as xt_pool,
            tc.tile_pool(name="wp", bufs=8) as w_pool,
            tc.tile_pool(name="op", bufs=4) as o_pool,
            tc.tile_pool(name="ps", bufs=1, space="PSUM") as ps_pool,
        ):
            for s in range(EPC):
                cap = int(caps[s])
                off = int(offs[s])
                mts = (cap + 127) // 128  # m-tiles in this slot
                xt_sb = xt_pool.tile([128, KC * cap], f32r, tag="xt", name=f"xt{s}")
                nc.gpsimd.dma_start(
                    xt_sb[:].rearrange("p (c t) -> p c t", c=KC),
                    xt[:, off:off + cap].rearrange("(c p) t -> p c t", p=128),
                )
                psums = {}
                for m in range(mts):
                    for n in range(NT):
                        psums[m, n] = ps_pool.tile(
                            [128, NW], mybir.dt.float32, tag=f"ps{m}{n}",
                            name=f"psum_{s}_{m}_{n}",
                        )
                for k in range(KC):
                    w_sb = w_pool.tile([128, DOUT], f32r, tag="w", name=f"w{s}_{k}")
                    nc.sync.dma_start(w_sb[:], w[s, k * 128:(k + 1) * 128, :])
                    for m in range(mts):
                        msz = min(128, cap - m * 128)
                        for n in range(NT):
                            nc.tensor.matmul(
                                psums[m, n][:msz],
                                xt_sb[:, k * cap + m * 128: k * cap + m * 128 + msz],
                                w_sb[:, n * NW:(n + 1) * NW],
                                start=(k == 0),
                                stop=(k == KC - 1),
                            )
                for m in range(mts):
                    msz = min(128, cap - m * 128)
                    o_sb = o_pool.tile([128, DOUT], mybir.dt.float32, tag="o",
                                       name=f"o_{s}_{m}")
                    for n in range(NT):
                        nc.vector.tensor_copy(
                            o_sb[:msz, n * NW:(n + 1) * NW], psums[m, n][:msz]
                        )
                    nc.scalar.dma_start(
                        out[off + m * 128: off + m * 128 + msz, :], o_sb[:msz]
                    )
    nc.compile()
    return nc


def _run(inputs, trace=False):
    x = np.asarray(inputs["input"], dtype=np.float32)
    w = np.ascontiguousarray(np.asarray(inputs["weight"], dtype=np.float32))
    counts = np.asarray(inputs["tokens_per_expert"], dtype=np.int64)
    starts = np.concatenate([[0], np.cumsum(counts)[:-1]])

    order = np.argsort(-counts, kind="stable")  # experts by size rank
    # slot s, core c -> expert order[s*NCORES + c]; capacity = rank-group max
    caps = tuple(
        int(np.ceil(max(1, counts[order[s * NCORES:(s + 1) * NCORES]].max()) / 32) * 32)
        for s in range(EPC)
    )
    offs = np.concatenate([[0], np.cumsum(caps)]).astype(int)
    sumcap = int(offs[-1])

    if caps not in _cache:
        _cache[caps] = _build(caps)
    nc = _cache[caps]

    in_maps = []
    for c in range(NCORES):
        xt_pack = np.zeros((DIN, sumcap), dtype=np.float32)
        w_pack = np.empty((EPC, DIN, DOUT), dtype=np.float32)
        for s in range(EPC):
            g = int(order[s * NCORES + c])
            cnt = int(counts[g])
            if cnt:
                xt_pack[:, offs[s]:offs[s] + cnt] = x[starts[g]:starts[g] + cnt].T
            w_pack[s] = w[g]
        in_maps.append({"xt": xt_pack, "w": w_pack})

    kw = {"trace_cores": list(range(NCORES))} if trace else {}
    res = run_bass_kernel_spmd(nc, in_maps, core_ids=list(range(NCORES)),
                               trace=trace, **kw)

    out = np.empty((T, DOUT), dtype=np.float32)
    for c in range(NCORES):
        for s in range(EPC):
            g = int(order[s * NCORES + c])
            cnt = int(counts[g])
            if cnt:
                out[starts[g]:starts[g] + cnt] = \
                    res.results[c]["out"][offs[s]:offs[s] + cnt]
    return out, res


def kernel(**inputs) -> np.ndarray:
    return _run(inputs)[0]


# revision 6
# speedup vs baseline: 1.9759x; 1.7398x over previous
"""Grouped GEMM (MoE routing) Trainium2 kernel.

Expert-parallel across 8 NeuronCores with size-sorted slot assignment:
experts are sorted by token count and slot s on every core holds the
experts of size-rank [8s, 8s+8), so one SPMD program with per-slot
capacities cap_s = roundup32(max count in rank group) serves all cores
with ~40% less padding than a fixed CAP=256.

Per slot: out[cap_s, DOUT] = x[cap_s, DIN] @ w[DIN, DOUT] on the PE in
bfloat16 (~2.6e-3 rel err), lhsT = host-transposed token
tiles, rhs = weight K-slabs [128, DOUT] streamed through SBUF,
accumulating over 20 K-chunks in PSUM ([128, 416] tiles, 4 DOUT chunks).
"""
import ml_dtypes
import numpy as np

import concourse.bass as bass
import concourse.mybir as mybir
import concourse.tile as tile
from concourse import bacc
from concourse.bass_utils import run_bass_kernel_spmd

G, T, DIN, DOUT = 64, 8192, 2560, 1664
NCORES = 8
EPC = G // NCORES   # expert slots per core
KC = DIN // 128     # 20 contraction chunks
NT = 4              # DOUT chunks
NW = DOUT // NT     # 416 (<=512 fp32 PSUM bank, >=256 for full-rate f32r)

_cache = {}


def _build(caps):
    offs = np.concatenate([[0], np.cumsum(caps)]).astype(int)
    sumcap = int(offs[-1])
    nc = bacc.Bacc(trn_type="TRN2", debug=False)
    bf16 = mybir.dt.bfloat16
    xt = nc.dram_tensor("xt", [DIN, sumcap], bf16, kind="ExternalInput").ap()
    w = nc.dram_tensor("w", [EPC, DIN, DOUT], bf16, kind="ExternalInput").ap()
    out = nc.dram_tensor(
        "out", [sumcap, DOUT], mybir.dt.float32, kind="ExternalOutput"
    ).ap()
    with tile.TileContext(nc) as tc:
        with (
            tc.tile_pool(name="xtp", bufs=2) as xt_pool,
            tc.tile_pool(name="wp", bufs=8) as w_pool,
            tc.tile_pool(name="op", bufs=4) as o_pool,
            tc.tile_pool(name="ps", bufs=1, space="PSUM") as ps_pool,
        ):
            for s in range(EPC):
                cap = int(caps[s])
                off = int(offs[s])
                mts = (cap + 127) // 128  # m-tiles in this slot
                xt_sb = xt_pool.tile([128, KC * cap], bf16, tag="xt", name=f"xt{s}")
                nc.gpsimd.dma_start(
                    xt_sb[:].rearrange("p (c t) -> p c t", c=KC),
                    xt[:, off:off + cap].rearrange("(c p) t -> p c t", p=128),
                )
                psums = {}
                for m in range(mts):
                    for n in range(NT):
                        psums[m, n] = ps_pool.tile(
                            [128, NW], mybir.dt.float32, tag=f"ps{m}{n}",
                            name=f"psum_{s}_{m}_{n}",
                        )
                for k in range(KC):
                    w_sb = w_pool.tile([128, DOUT], bf16, tag="w", name=f"w{s}_{k}")
                    nc.sync.dma_start(w_sb[:], w[s, k * 128:(k + 1) * 128, :])
                    for m in range(mts):
                        msz = min(128, cap - m * 128)
                        for n in range(NT):
                            nc.tensor.matmul(
                                psums[m, n][:msz],
                                xt_sb[:, k * cap + m * 128: k * cap + m * 128 + msz],
                                w_sb[:, n * NW:(n + 1) * NW],
                                start=(k == 0),
                                stop=(k == KC - 1),
                            )
                for m in range(mts):
                    msz = min(128, cap - m * 128)
                    o_sb = o_pool.tile([128, DOUT], mybir.dt.float32, tag="o",
                                       name=f"o_{s}_{m}")
                    for n in range(NT):
                        nc.vector.tensor_copy(
                            o_sb[:msz, n * NW:(n + 1) * NW], psums[m, n][:msz]
                        )
                    nc.scalar.dma_start(
                        out[off + m * 128: off + m * 128 + msz, :], o_sb[:msz]
                    )
    nc.compile()
    return nc


def _run(inputs, trace=False):
    x = np.asarray(inputs["input"], dtype=np.float32)
    w = np.ascontiguousarray(np.asarray(inputs["weight"], dtype=np.float32))
    counts = np.asarray(inputs["tokens_per_expert"], dtype=np.int64)
    starts = np.concatenate([[0], np.cumsum(counts)[:-1]])

    order = np.argsort(-counts, kind="stable")  # experts by size rank
    # slot s, core c -> expert order[s*NCORES + c]; capacity = rank-group max
    caps = tuple(
        int(np.ceil(max(1, counts[order[s * NCORES:(s + 1) * NCORES]].max()) / 32) * 32)
        for s in range(EPC)
    )
    offs = np.concatenate([[0], np.cumsum(caps)]).astype(int)
    sumcap = int(offs[-1])

    if caps not in _cache:
        _cache[caps] = _build(caps)
    nc = _cache[caps]

    in_maps = []
    for c in range(NCORES):
        xt_pack = np.zeros((DIN, sumcap), dtype=ml_dtypes.bfloat16)
        w_pack = np.empty((EPC, DIN, DOUT), dtype=ml_dtypes.bfloat16)
        for s in range(EPC):
            g = int(order[s * NCORES + c])
            cnt = int(counts[g])
            if cnt:
                xt_pack[:, offs[s]:offs[s] + cnt] = x[starts[g]:starts[g] + cnt].T
            w_pack[s] = w[g]
        in_maps.append({"xt": xt_pack, "w": w_pack})

    kw = {"trace_cores": list(range(NCORES))} if trace else {}
    res = run_bass_kernel_spmd(nc, in_maps, core_ids=list(range(NCORES)),
                               trace=trace, **kw)

    out = np.empty((T, DOUT), dtype=np.float32)
    for c in range(NCORES):
        for s in range(EPC):
            g = int(order[s * NCORES + c])
            cnt = int(counts[g])
            if cnt:
                out[starts[g]:starts[g] + cnt] = \
                    res.results[c]["out"][offs[s]:offs[s] + cnt]
    return out, res


def kernel(**inputs) -> np.ndarray:
    return _run(inputs)[0]


# revision 7
# speedup vs baseline: 1.9772x; 1.0006x over previous
"""Grouped GEMM (MoE routing) Trainium2 kernel.

Expert-parallel across 8 NeuronCores with size-sorted slot assignment:
experts are sorted by token count and slot s on every core holds the
experts of size-rank [8s, 8s+8), so one SPMD program with per-slot
capacities cap_s = roundup32(max count in rank group) serves all cores
with ~40% less padding than a fixed CAP=256.

Per slot: out[cap_s, DOUT] = x[cap_s, DIN] @ w[DIN, DOUT] on the PE in
bfloat16 (~2.6e-3 rel err), lhsT = host-transposed token
tiles, rhs = weight K-slabs [128, DOUT] streamed through SBUF,
accumulating over 20 K-chunks in PSUM ([128, 416] tiles, 4 DOUT chunks).
"""
import ml_dtypes
import numpy as np

import concourse.bass as bass
import concourse.mybir as mybir
import concourse.tile as tile
from concourse import bacc
from concourse.bass_utils import run_bass_kernel_spmd

G, T, DIN, DOUT = 64, 8192, 2560, 1664
NCORES = 8
EPC = G // NCORES   # expert slots per core
KC = DIN // 128     # 20 contraction chunks
NT = 4              # DOUT chunks
NW = DOUT // NT     # 416 (<=512 fp32 PSUM bank, >=256 for full-rate f32r)

_cache = {}


def _build(caps):
    offs = np.concatenate([[0], np.cumsum(caps)]).astype(int)
    sumcap = int(offs[-1])
    nc = bacc.Bacc(trn_type="TRN2", debug=False)
    bf16 = mybir.dt.bfloat16
    xt = nc.dram_tensor("xt", [DIN, sumcap], bf16, kind="ExternalInput").ap()
    w = nc.dram_tensor("w", [EPC, DIN, DOUT], bf16, kind="ExternalInput").ap()
    out = nc.dram_tensor(
        "out", [sumcap, DOUT], mybir.dt.float32, kind="ExternalOutput"
    ).ap()
    with tile.TileContext(nc) as tc:
        with (
            tc.tile_pool(name="xtp", bufs=3) as xt_pool,
            tc.tile_pool(name="wp", bufs=12) as w_pool,
            tc.tile_pool(name="op", bufs=4) as o_pool,
            tc.tile_pool(name="ps", bufs=1, space="PSUM") as ps_pool,
        ):
            for s in range(EPC):
                cap = int(caps[s])
                off = int(offs[s])
                mts = (cap + 127) // 128  # m-tiles in this slot
                xt_sb = xt_pool.tile([128, KC * cap], bf16, tag="xt", name=f"xt{s}")
                nc.gpsimd.dma_start(
                    xt_sb[:].rearrange("p (c t) -> p c t", c=KC),
                    xt[:, off:off + cap].rearrange("(c p) t -> p c t", p=128),
                )
                psums = {}
                for m in range(mts):
                    for n in range(NT):
                        psums[m, n] = ps_pool.tile(
                            [128, NW], mybir.dt.float32, tag=f"ps{m}{n}",
                            name=f"psum_{s}_{m}_{n}",
                        )
                for k in range(KC):
                    w_sb = w_pool.tile([128, DOUT], bf16, tag="w", name=f"w{s}_{k}")
                    nc.sync.dma_start(w_sb[:], w[s, k * 128:(k + 1) * 128, :])
                    for m in range(mts):
                        msz = min(128, cap - m * 128)
                        for n in range(NT):
                            nc.tensor.matmul(
                                psums[m, n][:msz],
                                xt_sb[:, k * cap + m * 128: k * cap + m * 128 + msz],
                                w_sb[:, n * NW:(n + 1) * NW],
                                start=(k == 0),
                                stop=(k == KC - 1),
                            )
                for m in range(mts):
                    msz = min(128, cap - m * 128)
                    o_sb = o_pool.tile([128, DOUT], mybir.dt.float32, tag="o",
                                       name=f"o_{s}_{m}")
                    for n in range(NT):
                        nc.vector.tensor_copy(
                            o_sb[:msz, n * NW:(n + 1) * NW], psums[m, n][:msz]
                        )
                    nc.scalar.dma_start(
                        out[off + m * 128: off + m * 128 + msz, :], o_sb[:msz]
                    )
    nc.compile()
    return nc


def _run(inputs, trace=False):
    x = np.asarray(inputs["input"], dtype=np.float32)
    w = np.ascontiguousarray(np.asarray(inputs["weight"], dtype=np.float32))
    counts = np.asarray(inputs["tokens_per_expert"], dtype=np.int64)
    starts = np.concatenate([[0], np.cumsum(counts)[:-1]])

    order = np.argsort(-counts, kind="stable")  # experts by size rank
    # slot s, core c -> expert order[s*NCORES + c]; capacity = rank-group max
    caps = tuple(
        int(np.ceil(max(1, counts[order[s * NCORES:(s + 1) * NCORES]].max()) / 32) * 32)
        for s in range(EPC)
    )
    offs = np.concatenate([[0], np.cumsum(caps)]).astype(int)
    sumcap = int(offs[-1])

    if caps not in _cache:
        _cache[caps] = _build(caps)
    nc = _cache[caps]

    in_maps = []
    for c in range(NCORES):
        xt_pack = np.zeros((DIN, sumcap), dtype=ml_dtypes.bfloat16)
        w_pack = np.empty((EPC, DIN, DOUT), dtype=ml_dtypes.bfloat16)
        for s in range(EPC):
            g = int(order[s * NCORES + c])
            cnt = int(counts[g])
            if cnt:
                xt_pack[:, offs[s]:offs[s] + cnt] = x[starts[g]:starts[g] + cnt].T
            w_pack[s] = w[g]
        in_maps.append({"xt": xt_pack, "w": w_pack})

    kw = {"trace_cores": list(range(NCORES))} if trace else {}
    res = run_bass_kernel_spmd(nc, in_maps, core_ids=list(range(NCORES)),
                               trace=trace, **kw)

    out = np.empty((T, DOUT), dtype=np.float32)
    for c in range(NCORES):
        for s in range(EPC):
            g = int(order[s * NCORES + c])
            cnt = int(counts[g])
            if cnt:
                out[starts[g]:starts[g] + cnt] = \
                    res.results[c]["out"][offs[s]:offs[s] + cnt]
    return out, res


def kernel(**inputs) -> np.ndarray:
    return _run(inputs)[0]
